# revision 1
# baseline (speedup 1.0000x reference)
"""Two-layer GCN (PyG GCNConv x2 + rrelu) on 8 Trainium2 NeuronCores.

Math: with A = adjacency-with-multiplicity + I (self loops), deg = in-degree
(including the self loop), dinv = deg^-1/2:
    z1[v] = dinv[v] * (sum_{u->v} dinv[u]*x[u]) @ W1 + b1
    g[u]  = dinv[u] * rrelu(z1[u])                      (dinv pre-folded for L2)
    z2[v] = dinv[v] * (sum_{u->v} g[u]) @ W2 + b2
Aggregation is linear, so the dense W matmul is applied post-aggregation on
the [128, 128] per-destination-block aggregate -- 128x less PE work than
transforming every edge message.

Sharding: destinations are range-sharded across the 8 cores (12544 each).
Every core keeps a replicated (dinv-prescaled, bf16) source-feature table in
its own HBM and fetches the source rows of its edges with dma_gather (int16
indices -> four even source windows; one call per (block, window), capped at
<=1008 indices by the 64-descriptor/engine SWDGE ring).  Per destination
block of 128 nodes, gathered edge-message chunks [128 edges, 128 feat] are
scatter-reduced on the TensorEngine by matmul with one-hot selectors
Sel[e, dest] = (d[e] == dest) generated on-device (is_equal with broadcast
operand).  Self-loop contributions bypass the gather: their source rows are
contiguous, so a plain DMA + identity matmul adds them.  Two NEFF dispatches
(layer 1, layer 2); the host transposes/concats activations between them.

The harness calls kernel(**inputs) with full inputs; index bucketing,
program build, compile, SPMD run on cores 0-7 and unshard all happen here.
"""

import sys

for _p in ("/opt/trn_rl_repo",):
    if _p not in sys.path:
        sys.path.insert(0, _p)

import numpy as np
import ml_dtypes

import concourse.bacc as bacc
import concourse.bass as bass
import concourse.mybir as mybir
import concourse.tile as tile
from concourse.bass_utils import run_bass_kernel_spmd

P = 128  # partition width == dest block width == feature width
RRELU_SLOPE = (1.0 / 8.0 + 1.0 / 3.0) / 2.0
MAX_CALL_COLS = 7   # dma_gather is capped at 1008 indices per call


class Cfg:
    def __init__(self, n_nodes, n_cores, blocks_per_core, superblock, in_f,
                 out1_f, out2_f, src_window, min_cap=1):
        self.n_nodes = n_nodes
        self.n_cores = n_cores
        self.bpc = blocks_per_core            # dest blocks per core
        self.sb = superblock                  # blocks per superblock
        assert blocks_per_core % superblock == 0
        self.sb_count = blocks_per_core // superblock
        self.in_f = in_f
        self.out1_f = out1_f
        self.out2_f = out2_f
        self.src_window = src_window          # int16 gather range per window
        self.min_cap = min_cap
        self.nodes_per_core = blocks_per_core * P
        self.n_pad = n_cores * self.nodes_per_core
        assert self.n_pad >= n_nodes
        assert src_window % P == 0 and src_window <= 32768
        self.n_chunks = -(-self.n_pad // src_window)
        self.tab_rows = self.n_chunks * src_window


FULL = Cfg(n_nodes=100000, n_cores=8, blocks_per_core=98, superblock=7,
           in_f=128, out1_f=128, out2_f=64, src_window=25088, min_cap=5)


def _call_plan(caps):
    """Per-block gather calls: (window k, col offset, n_cols), <=7 cols each."""
    plan = []
    for k, cap in enumerate(caps):
        c0 = 0
        while c0 < cap:
            n = min(MAX_CALL_COLS, cap - c0)
            plan.append((k, c0, n))
            c0 += n
    return plan


# --------------------------------------------------------------------------
# host-side index preprocessing
# --------------------------------------------------------------------------

def preprocess(edge_index, cfg):
    """Bucket edges by (dest block, src window); self loops are handled
    separately on-device.  Build per-core gather index / dest-local tables
    and the degree scaling."""
    row = edge_index[0].astype(np.int64)
    col = edge_index[1].astype(np.int64)
    n = cfg.n_nodes

    deg = np.bincount(col, minlength=cfg.n_pad).astype(np.float64) + 1.0
    dinv = (1.0 / np.sqrt(deg)).astype(np.float32)
    dinv[n:] = 1.0

    blk = col >> 7                      # global dest block
    chunk = row // cfg.src_window
    order = np.lexsort((chunk, blk))
    row, col, blk, chunk = row[order], col[order], blk[order], chunk[order]

    n_blocks = cfg.n_cores * cfg.bpc
    counts = np.zeros((n_blocks, cfg.n_chunks), dtype=np.int64)
    np.add.at(counts, (blk, chunk), 1)

    caps = np.maximum(-(-counts.max(axis=0) // P), cfg.min_cap).astype(np.int64)
    c_total = int(caps.sum())
    colbase = np.concatenate([[0], np.cumsum(caps)])[:-1]

    bc_start = np.zeros(n_blocks * cfg.n_chunks + 1, dtype=np.int64)
    np.cumsum(counts.reshape(-1), out=bc_start[1:])

    plan = _call_plan([int(x) for x in caps])
    per_core = []
    for c in range(cfg.n_cores):
        idx_parts = []
        d_tab = np.full((P, cfg.bpc * c_total), -1.0, dtype=np.float64)
        for b_loc in range(cfg.bpc):
            b_glob = c * cfg.bpc + b_loc
            segs = []
            for k in range(cfg.n_chunks):
                cap = int(caps[k])
                lo = bc_start[b_glob * cfg.n_chunks + k]
                hi = bc_start[b_glob * cfg.n_chunks + k + 1]
                cnt = hi - lo
                assert cnt <= cap * P, (cnt, cap * P)
                seg = np.zeros(cap * P, dtype=np.int64)
                seg[:cnt] = row[lo:hi] - k * cfg.src_window
                if cnt < cap * P:             # duplicate-pad (d stays -1)
                    seg[cnt:] = seg[0] if cnt > 0 else 0
                assert seg.min() >= 0 and seg.max() < cfg.src_window
                segs.append(seg)
                gcol0 = b_loc * c_total + colbase[k]
                d_seg = np.full(cap * P, -1.0)
                d_seg[:cnt] = (col[lo:hi] - b_glob * P).astype(np.float64)
                d_tab[:, gcol0:gcol0 + cap] = d_seg.reshape(cap, P).T
            for (k, c0, ncols) in plan:
                idx_parts.append(segs[k][c0 * P:(c0 + ncols) * P].astype(np.int16))
        idx_flat = [a.reshape(-1, 16).T for a in idx_parts]
        idx_tab = np.concatenate(idx_flat, axis=1)
        idx_tab = np.tile(idx_tab, (8, 1))          # [128, total/16]
        per_core.append({
            "idx_tab": np.ascontiguousarray(idx_tab),
            "d_tab": np.ascontiguousarray(d_tab.astype(ml_dtypes.bfloat16)),
            "dinv_sl": np.ascontiguousarray(
                dinv[c * cfg.nodes_per_core:(c + 1) * cfg.nodes_per_core]
            ).reshape(1, -1),
        })

    return {"caps": caps, "c_total": c_total, "dinv": dinv,
            "per_core": per_core}


# --------------------------------------------------------------------------
# bass program (one GCN layer, SPMD across cores; all data via inputs)
# --------------------------------------------------------------------------

def build_layer_program(cfg, caps, layer):
    """layer=1: out = bf16 gs1T [128, nodes_per_core]  (dinv*rrelu(z1), F-major)
       layer=2: out = f32  z2T  [out2_f, nodes_per_core]"""
    caps = [int(x) for x in caps]
    c_total = sum(caps)
    plan = _call_plan(caps)
    out_f = cfg.out1_f if layer == 1 else cfg.out2_f
    out_dt = mybir.dt.bfloat16 if layer == 1 else mybir.dt.float32
    idx_cols_blk = c_total * P // 16         # idx free-dim per block
    G = 8                                     # sel-gen chunk group width

    nc = bacc.Bacc("TRN2", target_bir_lowering=False, debug=False,
                   num_devices=cfg.n_cores,
                   num_swdge_queues=min(4, cfg.n_chunks))
    dt = mybir.dt
    src_tab = nc.dram_tensor("src_tab", [cfg.tab_rows, P], dt.bfloat16,
                             kind="ExternalInput")
    w_in = nc.dram_tensor("w", [P, out_f], dt.bfloat16, kind="ExternalInput")
    bias_in = nc.dram_tensor("bias", [out_f, 1], dt.float32, kind="ExternalInput")
    dinv_in = nc.dram_tensor("dinv_sl", [1, cfg.nodes_per_core], dt.float32,
                             kind="ExternalInput")
    idx_in = nc.dram_tensor("idx_tab", [P, cfg.bpc * idx_cols_blk], dt.int16,
                            kind="ExternalInput")
    d_in = nc.dram_tensor("d_tab", [P, cfg.bpc * c_total], dt.bfloat16,
                          kind="ExternalInput")
    iota_in = nc.dram_tensor("iota", [P, G * P], dt.bfloat16, kind="ExternalInput")
    ident_in = nc.dram_tensor("ident", [P, P], dt.bfloat16, kind="ExternalInput")
    ones_in = nc.dram_tensor("ones", [1, P], dt.float32, kind="ExternalInput")
    out_t = nc.dram_tensor("out_t", [out_f, cfg.nodes_per_core], out_dt,
                           kind="ExternalOutput")
    # per-core self-loop source rows, staged by the host (node-major slice of
    # src_tab rows owned by this core; avoids needing the core id on device)
    self_in = nc.dram_tensor("self_rows", [cfg.nodes_per_core, P], dt.bfloat16,
                             kind="ExternalInput")

    with tile.TileContext(nc) as tc:
        with (
            tc.tile_pool(name="const", bufs=1) as const_pool,
            tc.tile_pool(name="idx", bufs=2) as idx_pool,
            tc.tile_pool(name="msg", bufs=2) as msg_pool,
            tc.tile_pool(name="selfp", bufs=2) as self_pool,
            tc.tile_pool(name="sel", bufs=6) as sel_pool,
            tc.tile_pool(name="aggsb", bufs=3) as aggsb_pool,
            tc.tile_pool(name="tmp", bufs=3) as tmp_pool,
            tc.tile_pool(name="outsb", bufs=2) as out_pool,
            tc.tile_pool(name="psA", bufs=2, space="PSUM") as agg_psum,
            tc.tile_pool(name="psZ", bufs=2, space="PSUM") as z_psum,
            tc.tile_pool(name="psD", bufs=2, space="PSUM") as d_psum,
        ):
            w_sb = const_pool.tile([P, out_f], dt.bfloat16)
            nc.sync.dma_start(out=w_sb[:], in_=w_in[:])
            bias_sb = const_pool.tile([out_f, 1], dt.float32)
            nc.sync.dma_start(out=bias_sb[:], in_=bias_in[:])
            dinv_sb = const_pool.tile([1, cfg.nodes_per_core], dt.float32)
            nc.sync.dma_start(out=dinv_sb[:], in_=dinv_in[:])
            iota_sb = const_pool.tile([P, G * P], dt.bfloat16)
            nc.sync.dma_start(out=iota_sb[:], in_=iota_in[:])
            ident_sb = const_pool.tile([P, P], dt.bfloat16)
            nc.sync.dma_start(out=ident_sb[:], in_=ident_in[:])
            ones_sb = const_pool.tile([1, P], dt.float32)
            nc.sync.dma_start(out=ones_sb[:], in_=ones_in[:])
            d_sb = const_pool.tile([P, cfg.bpc * c_total], dt.bfloat16)
            nc.sync.dma_start(out=d_sb[:], in_=d_in[:])

            self_view = self_in.rearrange("(s b p) f -> s p b f",
                                          p=P, b=cfg.sb)

            for s in range(cfg.sb_count):
                idx_sb = idx_pool.tile([P, cfg.sb * idx_cols_blk], dt.int16)
                nc.sync.dma_start(
                    out=idx_sb[:],
                    in_=idx_in[:, s * cfg.sb * idx_cols_blk:
                               (s + 1) * cfg.sb * idx_cols_blk])
                # contiguous self-loop rows for this superblock
                selfs = self_pool.tile([P, cfg.sb, P], dt.bfloat16)
                nc.sync.dma_start(out=selfs[:], in_=self_view[s])

                msg = msg_pool.tile([P, cfg.sb * c_total, P], dt.bfloat16)
                off = 0
                for b7 in range(cfg.sb):
                    for (k, c0, ncols) in plan:
                        n_idx = ncols * P
                        mcol0 = b7 * c_total + colbase_val(caps, k) + c0
                        nc.gpsimd.dma_gather(
                            msg[:, mcol0:mcol0 + ncols, :],
                            src_tab[k * cfg.src_window:
                                    (k + 1) * cfg.src_window, :],
                            idx_sb[:, off:off + n_idx // 16],
                            n_idx, n_idx, P,
                            queue_num=k % 4,
                        )
                        off += n_idx // 16

                out_sb = out_pool.tile([out_f, cfg.sb * P], out_dt)
                for b7 in range(cfg.sb):
                    b_loc = s * cfg.sb + b7
                    dcol0 = b_loc * c_total
                    sels = []
                    done = 0
                    while done < c_total:
                        g = min(G, c_total - done)
                        sel = sel_pool.tile([P, G * P], dt.bfloat16)
                        nc.vector.tensor_tensor(
                            sel[:, :g * P],
                            iota_sb[:, :g * P],
                            d_sb[:, dcol0 + done:dcol0 + done + g]
                                .to_broadcast([P, g, P]),
                            mybir.AluOpType.is_equal,
                        )
                        sels.extend((sel, j) for j in range(g))
                        done += g

                    agg = agg_psum.tile([P, P], dt.float32)
                    for ci, (sel, j) in enumerate(sels):
                        nc.tensor.matmul(
                            agg[:],
                            lhsT=msg[:, b7 * c_total + ci, :],
                            rhs=sel[:, j * P:(j + 1) * P],
                            start=(ci == 0), stop=False,
                        )
                    # self-loop contribution: aggT += selfs[:, b7, :]^T
                    nc.tensor.matmul(
                        agg[:], lhsT=selfs[:, b7, :], rhs=ident_sb[:],
                        start=False, stop=True)

                    # dinv broadcast tile for this block (rank-1 matmul into
                    # psum, then to SBUF via the idle ScalarEngine -- DVE may
                    # read only one PSUM operand and zps is already PSUM)
                    dps = d_psum.tile([P, P], dt.float32)
                    nc.tensor.matmul(
                        dps[:], lhsT=ones_sb[:],
                        rhs=dinv_sb[:, b_loc * P:(b_loc + 1) * P],
                        start=True, stop=True)
                    dbc = aggsb_pool.tile([P, P], dt.float32, tag="dbc")
                    nc.scalar.copy(dbc[:], dps[:])

                    aggsb = aggsb_pool.tile([P, P], dt.bfloat16, tag="aggsb")
                    nc.vector.tensor_copy(aggsb[:], agg[:])

                    zps = z_psum.tile([out_f, P], dt.float32)
                    nc.tensor.matmul(zps[:], lhsT=w_sb[:], rhs=aggsb[:],
                                     start=True, stop=True)

                    o_sl = out_sb[:, b7 * P:(b7 + 1) * P]
                    bias_bc = bias_sb[:, 0:1].to_broadcast([out_f, P])
                    if layer == 1:
                        t1 = tmp_pool.tile([P, P], dt.float32, tag="t1")
                        nc.vector.tensor_tensor(t1[:], zps[:], dbc[:],
                                                mybir.AluOpType.mult)
                        u = tmp_pool.tile([P, P], dt.float32, tag="u")
                        nc.vector.tensor_tensor(u[:], t1[:], bias_bc,
                                                mybir.AluOpType.add)
                        rr = tmp_pool.tile([P, P], dt.float32, tag="rr")
                        nc.vector.scalar_tensor_tensor(
                            rr[:], u[:], float(RRELU_SLOPE), u[:],
                            mybir.AluOpType.mult, mybir.AluOpType.max)
                        nc.vector.tensor_tensor(o_sl, rr[:], dbc[:],
                                                mybir.AluOpType.mult)
                    else:
                        t1 = tmp_pool.tile([out_f, P], dt.float32, tag="t1")
                        nc.vector.tensor_tensor(t1[:], zps[:], dbc[:out_f, :],
                                                mybir.AluOpType.mult)
                        nc.vector.tensor_tensor(o_sl, t1[:], bias_bc,
                                                mybir.AluOpType.add)

                nc.sync.dma_start(
                    out=out_t[:, s * cfg.sb * P:(s + 1) * cfg.sb * P],
                    in_=out_sb[:])

    nc.compile()
    return nc


def colbase_val(caps, k):
    return int(sum(caps[:k]))


# --------------------------------------------------------------------------
# orchestration
# --------------------------------------------------------------------------

def _iota_tile(G=8):
    return np.tile(np.arange(P, dtype=np.float32), G)[None, :].repeat(P, 0).astype(ml_dtypes.bfloat16)


def _run_gcn(x, edge_index, W1, b1, W2, b2, cfg, runner=None, want_times=False):
    """Shared driver; runner(nc, in_maps) -> list of per-core output dicts."""
    meta = preprocess(np.asarray(edge_index), cfg)
    dinv = meta["dinv"]
    npc = cfg.nodes_per_core

    if runner is None:
        times = []

        def runner(nc, in_maps):
            r = run_bass_kernel_spmd(nc, in_maps, core_ids=list(range(cfg.n_cores)),
                                     trace=want_times)
            if want_times:
                times.append(r.exec_time_ns)
            return r.results
    else:
        times = None

    x = np.asarray(x, dtype=np.float32)
    xs = np.zeros((cfg.tab_rows, P), dtype=ml_dtypes.bfloat16)
    xs[:cfg.n_nodes] = (x * dinv[:cfg.n_nodes, None]).astype(ml_dtypes.bfloat16)

    iota = _iota_tile()
    ident = np.eye(P, dtype=np.float32).astype(ml_dtypes.bfloat16)
    ones = np.ones((1, P), np.float32)
    w1 = np.asarray(W1, np.float32).astype(ml_dtypes.bfloat16)
    w2 = np.asarray(W2, np.float32).astype(ml_dtypes.bfloat16)
    b1c = np.asarray(b1, np.float32).reshape(-1, 1)
    b2c = np.asarray(b2, np.float32).reshape(-1, 1)

    nc1 = build_layer_program(cfg, meta["caps"], layer=1)
    in_maps = [
        {"src_tab": xs, "w": w1, "bias": b1c, "iota": iota, "ident": ident,
         "ones": ones,
         "self_rows": np.ascontiguousarray(xs[c * npc:(c + 1) * npc]),
         **{k: pc[k] for k in ("idx_tab", "d_tab", "dinv_sl")}}
        for c, pc in enumerate(meta["per_core"])
    ]
    res1 = runner(nc1, in_maps)

    gs = np.zeros((cfg.tab_rows, P), dtype=ml_dtypes.bfloat16)
    for c in range(cfg.n_cores):
        gs[c * npc:(c + 1) * npc] = res1[c]["out_t"].T

    nc2 = build_layer_program(cfg, meta["caps"], layer=2)
    for c in range(cfg.n_cores):
        in_maps[c] = dict(in_maps[c])
        in_maps[c]["src_tab"] = gs
        in_maps[c]["self_rows"] = np.ascontiguousarray(gs[c * npc:(c + 1) * npc])
        in_maps[c]["w"] = w2
        in_maps[c]["bias"] = b2c
    res2 = runner(nc2, in_maps)

    out = np.zeros((cfg.n_pad, cfg.out2_f), dtype=np.float32)
    for c in range(cfg.n_cores):
        out[c * npc:(c + 1) * npc] = res2[c]["out_t"].T
    out = out[:cfg.n_nodes]
    if want_times and times is not None:
        return out, times
    return out


def kernel(x, edge_index, W1, b1, W2, b2):
    return _run_gcn(x, edge_index, W1, b1, W2, b2, FULL)



# revision 12
# speedup vs baseline: 1.0699x; 1.0699x over previous
"""Two-layer GCN (PyG GCNConv x2 + rrelu) on 8 Trainium2 NeuronCores.

Math: with A = adjacency-with-multiplicity + I (self loops), deg = in-degree
(including the self loop), dinv = deg^-1/2:
    z1[v] = dinv[v] * (sum_{u->v} dinv[u]*x[u]) @ W1 + b1
    g[u]  = dinv[u] * rrelu(z1[u])                      (dinv pre-folded for L2)
    z2[v] = dinv[v] * (sum_{u->v} g[u]) @ W2 + b2
Aggregation is linear, so the dense W matmul is applied post-aggregation on
the [128, 128] per-destination-block aggregate -- 128x less PE work than
transforming every edge message.

Sharding: destinations are range-sharded across the 8 cores (12544 each).
Every core keeps a replicated (dinv-prescaled, bf16) source-feature table in
its own HBM and fetches the source rows of its edges with dma_gather (int16
indices -> four 25088-row source windows).  The SWDGE descriptor-generation
fixed cost (~1us/call) dominated the previous version (one call per
(block, window) = 392/layer), so gathers are batched to ONE call per
(superblock of 7 blocks, window) = 56/layer with a window-major msg layout
so each call writes contiguous columns.

Per destination block of 128 nodes, gathered edge-message chunks
[128 edges, 128 feat] are scatter-reduced on the TensorEngine by matmul with
one-hot selectors Sel[e, dest] = (d[e] == dest) generated on-device
(is_equal with broadcast operand).  Self-loop contributions bypass the
gather: their source rows are contiguous, so a plain DMA + identity matmul
adds them.  The dense W matmul uses the f32->bf16-cast aggregate as the
STATIONARY operand so z comes out dest-major [dest, outF]; dinv[dest] is
then a per-partition scalar, letting the otherwise idle Scalar engine apply
the dinv scaling and rrelu (Prelu activation) off the Vector engine's
critical path.  Outputs are written dest-major = node-major, so the host
never transposes.  Two NEFF dispatches (layer 1, layer 2).

The harness calls kernel(**inputs) with full inputs; index bucketing,
program build, compile, SPMD run on cores 0-7 and unshard all happen here.
"""

import sys

for _p in ("/opt/trn_rl_repo",):
    if _p not in sys.path:
        sys.path.insert(0, _p)

import numpy as np
import ml_dtypes

import concourse.bacc as bacc
import concourse.bass as bass
import concourse.mybir as mybir
import concourse.tile as tile
from concourse.bass_utils import run_bass_kernel_spmd

P = 128  # partition width == dest block width == feature width
RRELU_SLOPE = (1.0 / 8.0 + 1.0 / 3.0) / 2.0
MAX_CALL_COLS = 7   # dma_gather is capped at 1008 indices per call


def _call_plan(caps, sb):
    """Per-superblock gather calls [(window, col0, ncols)], chunked to
    MAX_CALL_COLS and round-robin interleaved across windows so the four
    SWDGE queues fill evenly (a queue's ring holds only ~2 calls; emitting
    one window's calls back-to-back blocks GpSimd and starves the rest)."""
    pending = [(k, 0, sb * caps[k]) for k in range(len(caps))]
    plan = []
    while pending:
        nxt = []
        for (k, c0, total) in pending:
            ncols = min(MAX_CALL_COLS, total - c0)
            plan.append((k, c0, ncols))
            if c0 + ncols < total:
                nxt.append((k, c0 + ncols, total))
        pending = nxt
    return plan


class Cfg:
    def __init__(self, n_nodes, n_cores, blocks_per_core, superblock, in_f,
                 out1_f, out2_f, src_window, min_cap=1):
        self.n_nodes = n_nodes
        self.n_cores = n_cores
        self.bpc = blocks_per_core            # dest blocks per core
        self.sb = superblock                  # blocks per superblock
        assert blocks_per_core % superblock == 0
        self.sb_count = blocks_per_core // superblock
        self.in_f = in_f
        self.out1_f = out1_f
        self.out2_f = out2_f
        self.src_window = src_window          # int16 gather range per window
        self.min_cap = min_cap
        self.nodes_per_core = blocks_per_core * P
        self.n_pad = n_cores * self.nodes_per_core
        assert self.n_pad >= n_nodes
        assert src_window % P == 0 and src_window <= 32768
        self.n_chunks = -(-self.n_pad // src_window)
        self.tab_rows = self.n_chunks * src_window


FULL = Cfg(n_nodes=100000, n_cores=8, blocks_per_core=98, superblock=7,
           in_f=128, out1_f=128, out2_f=64, src_window=25088, min_cap=4)


# --------------------------------------------------------------------------
# host-side index preprocessing
# --------------------------------------------------------------------------

def preprocess(edge_index, cfg):
    """Bucket edges by (dest block, src window); self loops are handled
    separately on-device.  Build per-core gather index / dest-local tables
    and the degree scaling.  Index/msg columns are laid out window-major
    within each superblock so one dma_gather call per (superblock, window)
    writes a contiguous column range."""
    row = edge_index[0].astype(np.int64)
    col = edge_index[1].astype(np.int64)
    n = cfg.n_nodes

    deg = np.bincount(col, minlength=cfg.n_pad).astype(np.float64) + 1.0
    dinv = (1.0 / np.sqrt(deg)).astype(np.float32)
    dinv[n:] = 1.0

    blk = col >> 7                      # global dest block
    chunk = row // cfg.src_window
    order = np.lexsort((chunk, blk))
    row, col, blk, chunk = row[order], col[order], blk[order], chunk[order]

    n_blocks = cfg.n_cores * cfg.bpc
    counts = np.zeros((n_blocks, cfg.n_chunks), dtype=np.int64)
    np.add.at(counts, (blk, chunk), 1)

    caps = np.maximum(-(-counts.max(axis=0) // P), cfg.min_cap).astype(np.int64)
    c_total = int(caps.sum())
    cumcaps = np.concatenate([[0], np.cumsum(caps)])  # window col bases

    bc_start = np.zeros(n_blocks * cfg.n_chunks + 1, dtype=np.int64)
    np.cumsum(counts.reshape(-1), out=bc_start[1:])

    per_core = []
    for c in range(cfg.n_cores):
        # idx values per (block, window): [bpc][n_chunks] -> arrays of cap*P
        segs = [[None] * cfg.n_chunks for _ in range(cfg.bpc)]
        d_tab = np.full((P, cfg.bpc * c_total), -1.0, dtype=np.float64)
        for b_loc in range(cfg.bpc):
            b_glob = c * cfg.bpc + b_loc
            for k in range(cfg.n_chunks):
                cap = int(caps[k])
                lo = bc_start[b_glob * cfg.n_chunks + k]
                hi = bc_start[b_glob * cfg.n_chunks + k + 1]
                cnt = hi - lo
                assert cnt <= cap * P, (cnt, cap * P)
                seg = np.zeros(cap * P, dtype=np.int64)
                seg[:cnt] = row[lo:hi] - k * cfg.src_window
                if cnt < cap * P:             # duplicate-pad (d stays -1)
                    seg[cnt:] = seg[0] if cnt > 0 else 0
                assert seg.min() >= 0 and seg.max() < cfg.src_window
                segs[b_loc][k] = seg
                # d_tab col order per block: (window, cap-col) == msg order
                gcol0 = b_loc * c_total + int(cumcaps[k])
                d_seg = np.full(cap * P, -1.0)
                d_seg[:cnt] = (col[lo:hi] - b_glob * P).astype(np.float64)
                d_tab[:, gcol0:gcol0 + cap] = d_seg.reshape(cap, P).T
        # idx table in call-emission order (round-robin interleaved over
        # windows so all SWDGE queues stay fed -- see _call_plan)
        plan = _call_plan([int(x) for x in caps], cfg.sb)
        idx_parts = []
        for s in range(cfg.sb_count):
            win_flat = []  # per window: concatenated idx of the superblock
            for k in range(cfg.n_chunks):
                win_flat.append(np.concatenate(
                    [segs[s * cfg.sb + b7][k] for b7 in range(cfg.sb)]))
            for (k, c0, ncols) in plan:
                idx_parts.append(
                    win_flat[k][c0 * P:(c0 + ncols) * P].astype(np.int16))
        idx_flat = [a.reshape(-1, 16).T for a in idx_parts]
        idx_tab = np.concatenate(idx_flat, axis=1)
        idx_tab = np.tile(idx_tab, (8, 1))          # [128, total/16]
        # dinv columns: dinv_cols[p, b] = dinv[core_base + b*128 + p]
        dslice = dinv[c * cfg.nodes_per_core:(c + 1) * cfg.nodes_per_core]
        dinv_cols = np.ascontiguousarray(dslice.reshape(cfg.bpc, P).T)
        per_core.append({
            "idx_tab": np.ascontiguousarray(idx_tab),
            "d_tab": np.ascontiguousarray(d_tab.astype(ml_dtypes.bfloat16)),
            "dinv_cols": dinv_cols,
        })

    return {"caps": caps, "c_total": c_total, "dinv": dinv,
            "per_core": per_core}


# --------------------------------------------------------------------------
# bass program (one GCN layer, SPMD across cores; all data via inputs)
# --------------------------------------------------------------------------

def build_layer_program(cfg, caps, layer, has_bias=False):
    """layer=1: out = bf16 g [nodes_per_core, 128]  (dinv*rrelu(z1), node-major)
       layer=2: out = f32  z2 [nodes_per_core, out2_f]"""
    caps = [int(x) for x in caps]
    c_total = sum(caps)
    cumcaps = [0]
    for cp in caps:
        cumcaps.append(cumcaps[-1] + cp)
    plan = _call_plan(caps, cfg.sb)
    out_f = cfg.out1_f if layer == 1 else cfg.out2_f
    out_dt = mybir.dt.bfloat16 if layer == 1 else mybir.dt.float32
    idx_cols_blk = c_total * P // 16         # idx free-dim per block
    G = 8                                     # sel-gen chunk group width

    nc = bacc.Bacc("TRN2", target_bir_lowering=False, debug=False,
                   num_devices=cfg.n_cores,
                   num_swdge_queues=min(4, cfg.n_chunks))
    dt = mybir.dt
    src_tab = nc.dram_tensor("src_tab", [cfg.tab_rows, P], dt.bfloat16,
                             kind="ExternalInput")
    w_in = nc.dram_tensor("w", [P, out_f], dt.bfloat16, kind="ExternalInput")
    dinv_in = nc.dram_tensor("dinv_cols", [P, cfg.bpc], dt.float32,
                             kind="ExternalInput")
    idx_in = nc.dram_tensor("idx_tab", [P, cfg.bpc * idx_cols_blk], dt.int16,
                            kind="ExternalInput")
    d_in = nc.dram_tensor("d_tab", [P, cfg.bpc * c_total], dt.bfloat16,
                          kind="ExternalInput")
    iota_in = nc.dram_tensor("iota", [P, G * P], dt.bfloat16, kind="ExternalInput")
    ident_in = nc.dram_tensor("ident", [P, P], dt.bfloat16, kind="ExternalInput")
    out_t = nc.dram_tensor("out_t", [cfg.nodes_per_core, out_f], out_dt,
                           kind="ExternalOutput")
    # per-core self-loop source rows, staged by the host (node-major slice of
    # src_tab rows owned by this core; avoids needing the core id on device)
    self_in = nc.dram_tensor("self_rows", [cfg.nodes_per_core, P], dt.bfloat16,
                             kind="ExternalInput")
    if has_bias:
        bias_in = nc.dram_tensor("bias_full", [P, out_f], dt.float32,
                                 kind="ExternalInput")

    with tile.TileContext(nc) as tc:
        with (
            tc.tile_pool(name="const", bufs=1) as const_pool,
            tc.tile_pool(name="idx", bufs=3) as idx_pool,
            tc.tile_pool(name="msg", bufs=3) as msg_pool,
            tc.tile_pool(name="selfp", bufs=3) as self_pool,
            tc.tile_pool(name="sel", bufs=6) as sel_pool,
            tc.tile_pool(name="aggsb", bufs=3) as aggsb_pool,
            tc.tile_pool(name="tmp", bufs=3) as tmp_pool,
            tc.tile_pool(name="outsb", bufs=2) as out_pool,
            tc.tile_pool(name="psA", bufs=3, space="PSUM") as agg_psum,
            tc.tile_pool(name="psZ", bufs=2, space="PSUM") as z_psum,
        ):
            w_sb = const_pool.tile([P, out_f], dt.bfloat16)
            nc.sync.dma_start(out=w_sb[:], in_=w_in[:])
            dinv_sb = const_pool.tile([P, cfg.bpc], dt.float32)
            nc.sync.dma_start(out=dinv_sb[:], in_=dinv_in[:])
            iota_sb = const_pool.tile([P, G * P], dt.bfloat16)
            nc.sync.dma_start(out=iota_sb[:], in_=iota_in[:])
            ident_sb = const_pool.tile([P, P], dt.bfloat16)
            nc.sync.dma_start(out=ident_sb[:], in_=ident_in[:])
            d_sb = const_pool.tile([P, cfg.bpc * c_total], dt.bfloat16)
            nc.sync.dma_start(out=d_sb[:], in_=d_in[:])
            alpha_sb = const_pool.tile([P, 1], dt.float32)
            nc.vector.memset(alpha_sb[:], float(RRELU_SLOPE))
            if has_bias:
                bias_sb = const_pool.tile([P, out_f], dt.float32)
                nc.sync.dma_start(out=bias_sb[:], in_=bias_in[:])

            self_view = self_in.rearrange("(s b p) f -> s p b f",
                                          p=P, b=cfg.sb)
            out_view = out_t.rearrange("(s b p) f -> s p b f",
                                       p=P, b=cfg.sb)

            def finish_block(b_loc, aggsb, out_sb, b7):
                """W matmul (agg stationary -> z dest-major) + ACT epilogue."""
                zps = z_psum.tile([P, out_f], dt.float32)
                nc.tensor.matmul(zps[:], lhsT=aggsb[:], rhs=w_sb[:],
                                 start=True, stop=True)
                dcol = dinv_sb[:, b_loc:b_loc + 1]
                o_sl = out_sb[:, b7, :]
                if layer == 1:
                    t1 = tmp_pool.tile([P, out_f], dt.float32, tag="t1")
                    if has_bias:
                        tz = tmp_pool.tile([P, out_f], dt.float32, tag="tz")
                        nc.scalar.activation(
                            tz[:], zps[:], mybir.ActivationFunctionType.Copy,
                            scale=dcol)
                        tb = tmp_pool.tile([P, out_f], dt.float32, tag="tb")
                        nc.vector.tensor_tensor(tb[:], tz[:], bias_sb[:],
                                                mybir.AluOpType.add)
                        nc.scalar.activation(
                            t1[:], tb[:], mybir.ActivationFunctionType.Prelu,
                            scale=1.0, alpha=alpha_sb[:, 0:1])
                    else:
                        nc.scalar.activation(
                            t1[:], zps[:], mybir.ActivationFunctionType.Prelu,
                            scale=dcol, alpha=alpha_sb[:, 0:1])
                    nc.scalar.activation(
                        o_sl, t1[:], mybir.ActivationFunctionType.Copy,
                        scale=dcol)
                else:
                    if has_bias:
                        tz = tmp_pool.tile([P, out_f], dt.float32, tag="tz")
                        nc.scalar.activation(
                            tz[:], zps[:], mybir.ActivationFunctionType.Copy,
                            scale=dcol)
                        nc.vector.tensor_tensor(o_sl, tz[:], bias_sb[:],
                                                mybir.AluOpType.add)
                    else:
                        nc.scalar.activation(
                            o_sl, zps[:], mybir.ActivationFunctionType.Copy,
                            scale=dcol)

            for s in range(cfg.sb_count):
                idx_sb = idx_pool.tile([P, cfg.sb * idx_cols_blk], dt.int16)
                nc.sync.dma_start(
                    out=idx_sb[:],
                    in_=idx_in[:, s * cfg.sb * idx_cols_blk:
                               (s + 1) * cfg.sb * idx_cols_blk])
                # contiguous self-loop rows for this superblock
                selfs = self_pool.tile([P, cfg.sb, P], dt.bfloat16)
                nc.sync.dma_start(out=selfs[:], in_=self_view[s])

                # gather calls per window (window-major msg cols), chunked to
                # MAX_CALL_COLS columns and interleaved across queues
                msg = msg_pool.tile([P, cfg.sb * c_total, P], dt.bfloat16)
                off = 0
                for (k, c0, ncols) in plan:
                    mcol0 = cfg.sb * cumcaps[k] + c0
                    n_idx = ncols * P
                    nc.gpsimd.dma_gather(
                        msg[:, mcol0:mcol0 + ncols, :],
                        src_tab[k * cfg.src_window:
                                (k + 1) * cfg.src_window, :],
                        idx_sb[:, off:off + n_idx // 16],
                        n_idx, n_idx, P,
                        queue_num=k % 4,
                    )
                    off += n_idx // 16

                out_sb = out_pool.tile([P, cfg.sb, out_f], out_dt)
                pending = None  # (b_loc, aggsb, b7) 1-deep pipeline
                for b7 in range(cfg.sb):
                    b_loc = s * cfg.sb + b7
                    dcol0 = b_loc * c_total
                    sels = []
                    done = 0
                    while done < c_total:
                        g = min(G, c_total - done)
                        sel = sel_pool.tile([P, G * P], dt.bfloat16)
                        nc.vector.tensor_tensor(
                            sel[:, :g * P],
                            iota_sb[:, :g * P],
                            d_sb[:, dcol0 + done:dcol0 + done + g]
                                .to_broadcast([P, g, P]),
                            mybir.AluOpType.is_equal,
                        )
                        sels.extend((sel, j) for j in range(g))
                        done += g

                    agg = agg_psum.tile([P, P], dt.float32)
                    for ci, (sel, j) in enumerate(sels):
                        # msg col of (block b7, window k, col c) enumerated in
                        # the same (k, c) order as d_tab columns
                        k = 0
                        while ci >= cumcaps[k + 1]:
                            k += 1
                        mcol = cfg.sb * cumcaps[k] + b7 * caps[k] + (ci - cumcaps[k])
                        nc.tensor.matmul(
                            agg[:],
                            lhsT=msg[:, mcol, :],
                            rhs=sel[:, j * P:(j + 1) * P],
                            start=(ci == 0), stop=False,
                        )
                    # self-loop contribution: aggT += selfs[:, b7, :]^T
                    nc.tensor.matmul(
                        agg[:], lhsT=selfs[:, b7, :], rhs=ident_sb[:],
                        start=False, stop=True)

                    aggsb = aggsb_pool.tile([P, P], dt.bfloat16)
                    nc.vector.tensor_copy(aggsb[:], agg[:])

                    if pending is not None:
                        finish_block(*pending)
                    pending = (b_loc, aggsb, out_sb, b7)
                finish_block(*pending)

                nc.sync.dma_start(out=out_view[s], in_=out_sb[:])

    nc.compile()
    return nc


# --------------------------------------------------------------------------
# orchestration
# --------------------------------------------------------------------------

def _iota_tile(G=8):
    return np.tile(np.arange(P, dtype=np.float32), G)[None, :].repeat(P, 0).astype(ml_dtypes.bfloat16)


def _run_gcn(x, edge_index, W1, b1, W2, b2, cfg, runner=None, want_times=False):
    """Shared driver; runner(nc, in_maps) -> list of per-core output dicts."""
    meta = preprocess(np.asarray(edge_index), cfg)
    dinv = meta["dinv"]
    npc = cfg.nodes_per_core

    if runner is None:
        times = []

        def runner(nc, in_maps):
            r = run_bass_kernel_spmd(nc, in_maps, core_ids=list(range(cfg.n_cores)),
                                     trace=want_times)
            if want_times:
                times.append(r.exec_time_ns)
            return r.results
    else:
        times = None

    x = np.asarray(x, dtype=np.float32)
    xs = np.zeros((cfg.tab_rows, P), dtype=ml_dtypes.bfloat16)
    xs[:cfg.n_nodes] = (x * dinv[:cfg.n_nodes, None]).astype(ml_dtypes.bfloat16)

    iota = _iota_tile()
    ident = np.eye(P, dtype=np.float32).astype(ml_dtypes.bfloat16)
    w1 = np.asarray(W1, np.float32).astype(ml_dtypes.bfloat16)
    w2 = np.asarray(W2, np.float32).astype(ml_dtypes.bfloat16)
    b1c = np.asarray(b1, np.float32).reshape(-1)
    b2c = np.asarray(b2, np.float32).reshape(-1)
    hb1 = bool(np.any(b1c != 0.0))
    hb2 = bool(np.any(b2c != 0.0))

    nc1 = build_layer_program(cfg, meta["caps"], layer=1, has_bias=hb1)
    in_maps = [
        {"src_tab": xs, "w": w1, "iota": iota, "ident": ident,
         "self_rows": np.ascontiguousarray(xs[c * npc:(c + 1) * npc]),
         **{k: pc[k] for k in ("idx_tab", "d_tab", "dinv_cols")}}
        for c, pc in enumerate(meta["per_core"])
    ]
    if hb1:
        bf = np.ascontiguousarray(np.broadcast_to(b1c, (P, cfg.out1_f)).astype(np.float32))
        for m in in_maps:
            m["bias_full"] = bf
    res1 = runner(nc1, in_maps)

    gs = np.zeros((cfg.tab_rows, P), dtype=ml_dtypes.bfloat16)
    for c in range(cfg.n_cores):
        gs[c * npc:(c + 1) * npc] = res1[c]["out_t"]

    nc2 = build_layer_program(cfg, meta["caps"], layer=2, has_bias=hb2)
    for c in range(cfg.n_cores):
        in_maps[c] = dict(in_maps[c])
        in_maps[c]["src_tab"] = gs
        in_maps[c]["self_rows"] = np.ascontiguousarray(gs[c * npc:(c + 1) * npc])
        in_maps[c]["w"] = w2
        in_maps[c].pop("bias_full", None)
        if hb2:
            in_maps[c]["bias_full"] = np.ascontiguousarray(
                np.broadcast_to(b2c, (P, cfg.out2_f)).astype(np.float32))
    res2 = runner(nc2, in_maps)

    out = np.zeros((cfg.n_pad, cfg.out2_f), dtype=np.float32)
    for c in range(cfg.n_cores):
        out[c * npc:(c + 1) * npc] = res2[c]["out_t"]
    out = out[:cfg.n_nodes]
    if want_times and times is not None:
        return out, times
    return out


def kernel(x, edge_index, W1, b1, W2, b2):
    return _run_gcn(x, edge_index, W1, b1, W2, b2, FULL)
